# revision 3
# baseline (speedup 1.0000x reference)
"""CenterLoss Trainium2 kernel (8 NeuronCores, SPMD).

Strategy (class-sharded scatter):
  - Algebraic reduction: with xsum[c] = sum of x rows labeled c and
    counts[c] = #rows labeled c,
        segment_sum(centers[labels] - x)[c] = counts[c]*centers[c] - xsum[c]
        loss = (sum(x^2) - 2*sum(xsum*centers) + sum(counts*|centers|^2))/2/B
    so the device only needs xsum, and sum(x^2); no centers gather at all.
  - Host computes (cheap, O(B) int work) the sorted row order; each core owns a
    contiguous window of 125 classes and index-gathers its own rows from HBM
    (data-dependent scatter -> gather conversion). Each 128-row tile then hits
    a <=128-wide class window, so the per-class sums collapse to ONE one-hot
    matmul per tile accumulated into a single PSUM bank.
  - Per tile: indirect DMA gathers 128 rows (cast f32->bf16 in the DMA);
    DVE builds the one-hot mask (iota == label) in bf16; PE accumulates
    mask^T @ x_tile into PSUM[128cls, 512]; ACT squares+reduces rows for
    sum(x^2). Host does the final tiny O(C*D) update in float64.
"""

import numpy as np
import ml_dtypes

import concourse.bass as bass
import concourse.mybir as mybir
from concourse.bass import IndirectOffsetOnAxis
from concourse.bass_utils import run_bass_kernel_spmd

P = 128
D = 512
C = 1000
NCORES = 8
W = C // NCORES          # 125 classes per core
NB = 8                   # rotating mask buffers

f32 = mybir.dt.float32
bf16 = mybir.dt.bfloat16
i32 = mybir.dt.int32

GATHER_CAST = True       # cast f32->bf16 inside the gather DMA

_kernel_cache = {}
_last_results = None     # test.py introspection
TRACE = False


def _build(T, nrows, reps=1):
    nc = bass.Bass()
    x_d = nc.declare_dram_parameter("x", [nrows, D], f32, isOutput=False)
    iota_d = nc.declare_dram_parameter("iota", [P, P], f32, isOutput=False)
    lab_d = nc.declare_dram_parameter("labels_rel", [P, T], f32, isOutput=False)
    idx_d = nc.declare_dram_parameter("idx", [P, T], i32, isOutput=False)
    xsum_d = nc.declare_dram_parameter("xsum", [P, D], f32, isOutput=True)
    x2_d = nc.declare_dram_parameter("x2", [P, T], f32, isOutput=True)

    xg_dt = bf16 if GATHER_CAST else f32

    with (
        nc.sbuf_tensor([P, T * D], xg_dt) as xg,
        nc.sbuf_tensor([P, NB * D], bf16) as xgb,
        nc.sbuf_tensor([P, NB * P], bf16) as masks,
        nc.sbuf_tensor([P, P], f32) as iota_sb,
        nc.sbuf_tensor([P, T], f32) as lab_sb,
        nc.sbuf_tensor([P, T], i32) as idx_sb,
        nc.sbuf_tensor([P, D], f32) as xsum_sb,
        nc.sbuf_tensor([P, T], f32) as x2cols,
        nc.sbuf_tensor([P, D], f32) as sq_scratch,
        nc.psum_tensor([P, D], f32) as psum,
        nc.semaphore("init_sem") as init_sem,
        nc.semaphore("g_sem") as g_sem,
        nc.semaphore("m_sem") as m_sem,
        nc.semaphore("pe_sem") as pe_sem,
        nc.semaphore("act_sem") as act_sem,
        nc.semaphore("dve_done") as dve_done,
        nc.semaphore("out_sem") as out_sem,
        nc.Block() as block,
    ):
        @block.sync
        def _(sync):
            sync.dma_start(out=idx_sb[:], in_=idx_d[:]).then_inc(init_sem, 16)
            sync.dma_start(out=iota_sb[:], in_=iota_d[:]).then_inc(init_sem, 16)
            sync.dma_start(out=lab_sb[:], in_=lab_d[:]).then_inc(init_sem, 16)
            sync.wait_ge(dve_done, 1)
            sync.dma_start(out=xsum_d[:], in_=xsum_sb[:]).then_inc(out_sem, 16)
            sync.wait_ge(act_sem, reps * T)
            sync.dma_start(out=x2_d[:], in_=x2cols[:]).then_inc(out_sem, 16)
            sync.wait_ge(out_sem, 32)

        @block.gpsimd
        def _(gpsimd):
            gpsimd.wait_ge(init_sem, 48)
            for r in range(reps):
                if r > 0:
                    gpsimd.wait_ge(pe_sem, r * T)
                    gpsimd.wait_ge(act_sem, r * T)
                for t in range(T):
                    gpsimd.indirect_dma_start(
                        out=xg[:, t * D:(t + 1) * D],
                        out_offset=None,
                        in_=x_d[:],
                        in_offset=IndirectOffsetOnAxis(ap=idx_sb[:, t:t + 1], axis=0),
                    ).then_inc(g_sem, 16)

        @block.vector
        def _(vector):
            vector.wait_ge(init_sem, 48)
            for r in range(reps):
                for t in range(T):
                    i = r * T + t
                    if i >= NB:
                        vector.wait_ge(pe_sem, i - NB + 1)
                    if not GATHER_CAST:
                        vector.wait_ge(g_sem, 16 * (i + 1))
                        vector.tensor_copy(
                            out=xgb[:, (t % NB) * D:(t % NB + 1) * D],
                            in_=xg[:, t * D:(t + 1) * D],
                        )
                    vector.tensor_scalar(
                        out=masks[:, (i % NB) * P:(i % NB + 1) * P],
                        in0=iota_sb[:],
                        scalar1=lab_sb[:, t:t + 1],
                        scalar2=None,
                        op0=mybir.AluOpType.is_equal,
                    ).then_inc(m_sem, 1)
            vector.wait_ge(pe_sem, reps * T)
            vector.tensor_copy(out=xsum_sb[:], in_=psum[:]).then_inc(dve_done, 1)

        @block.tensor
        def _(tensor):
            for r in range(reps):
                for t in range(T):
                    i = r * T + t
                    tensor.wait_ge(m_sem, i + 1)
                    tensor.wait_ge(g_sem, 16 * (i + 1))
                    rhs = (xg[:, t * D:(t + 1) * D] if GATHER_CAST
                           else xgb[:, (t % NB) * D:(t % NB + 1) * D])
                    tensor.matmul(
                        out=psum[:],
                        lhsT=masks[:, (i % NB) * P:(i % NB + 1) * P],
                        rhs=rhs,
                        start=(t == 0),
                        stop=(t == T - 1),
                    ).then_inc(pe_sem, 1)

        @block.scalar
        def _(scalar):
            for r in range(reps):
                for t in range(T):
                    i = r * T + t
                    scalar.wait_ge(g_sem, 16 * (i + 1))
                    scalar.activation(
                        out=sq_scratch[:],
                        in_=xg[:, t * D:(t + 1) * D],
                        func=mybir.ActivationFunctionType.Square,
                        accum_out=x2cols[:, t:t + 1],
                    ).then_inc(act_sem, 1)

    return nc


def kernel(x, labels, centers):
    global _last_results
    x = np.ascontiguousarray(np.asarray(x), dtype=np.float32)
    labels = np.asarray(labels)
    assert labels.dtype == np.int32
    centers = np.asarray(centers, dtype=np.float32)
    B = x.shape[0]

    # ---- host: index preprocessing (sort rows by class, window per core) ----
    order = np.argsort(labels, kind="stable")
    counts = np.bincount(labels, minlength=C)
    bounds = np.concatenate([[0], np.cumsum(counts)])
    core_rows = [np.sort(order[bounds[c * W]:bounds[(c + 1) * W]])
                 for c in range(NCORES)]
    T = max(1, max((len(r) + P - 1) // P for r in core_rows))

    key = (T, B)
    if key not in _kernel_cache:
        _kernel_cache[key] = _build(T, B)
    nc = _kernel_cache[key]

    iota_np = np.tile(np.arange(P, dtype=np.float32), (P, 1))
    in_maps = []
    n_dummy = []
    for c in range(NCORES):
        rows = core_rows[c]
        n = len(rows)
        pad = T * P - n
        n_dummy.append(pad)
        rows_p = np.concatenate([rows, np.zeros(pad, dtype=np.int64)])
        lab_rel = np.concatenate([
            labels[rows].astype(np.int64) - c * W,
            np.full(pad, -1, dtype=np.int64),
        ])
        in_maps.append({
            "x": x,
            "iota": iota_np,
            "labels_rel": lab_rel.reshape(T, P).T.astype(np.float32).copy(),
            "idx": rows_p.reshape(T, P).T.astype(np.int32).copy(),
        })

    res = run_bass_kernel_spmd(nc, in_maps, list(range(NCORES)), trace=TRACE)
    _last_results = res

    # ---- host: finalize in float64 ----
    xsum = np.concatenate(
        [res.results[c]["xsum"][:W].astype(np.float64) for c in range(NCORES)],
        axis=0,
    )  # [C, D]

    if GATHER_CAST:
        x0 = x[0].astype(ml_dtypes.bfloat16).astype(np.float64)
    else:
        x0 = x[0].astype(np.float64)
    s0 = float((x0 * x0).sum())
    sum_x2 = sum(float(res.results[c]["x2"].astype(np.float64).sum())
                 for c in range(NCORES))
    sum_x2 -= s0 * sum(n_dummy)

    cnt = counts.astype(np.float64)
    c64 = centers.astype(np.float64)
    sums_diff = cnt[:, None] * c64 - xsum
    mean_diff = sums_diff / np.maximum(cnt, 1.0)[:, None]
    update = np.where(cnt[:, None] > 0, mean_diff * 0.5, 0.0)
    new_centers = (c64 - update).astype(np.float32)

    loss64 = (sum_x2 - 2.0 * float((xsum * c64).sum())
              + float((cnt * (c64 * c64).sum(axis=1)).sum())) / 2.0 / B
    loss = np.float32(loss64)
    return loss, new_centers
